# revision 6
# baseline (speedup 1.0000x reference)
"""ENLCA Performer linear-attention kernel, distributed over 8 TRN2 NeuronCores.

Sharding: data-parallel over batch N=16 -> 2 images per core (attention is
independent per image except for the global key-feature max, which is a
scalar all-reduce-max across cores, done with lax.pmax inside the pmapped
program so the whole computation including the collective runs on-device).

Shapes are hardcoded per the problem spec:
  x [16,128,128,128] f32, w1/w2 [64,128], b1/b2 [64], wa [128,128], ba [128],
  proj [128,64].
"""

import numpy as np
import jax
import jax.numpy as jnp
from functools import partial

K_AMP = 6.0 ** 0.5
RES_SCALE = 0.1
EPS_NORM = 5e-05
EPS_KERN = 1e-4
N_DEV = 8


def _l2norm(t):
    n = jnp.linalg.norm(t, axis=-1, keepdims=True)
    return t / jnp.maximum(n, EPS_NORM)


@partial(
    jax.pmap,
    axis_name="dp",
    in_axes=(0, None, None, None, None, None),
)
def _shard_fn(x, wcat, b1, b2, ba, proj):
    # x: [2, C, H, W] on each of the 8 cores
    n, C, H, W = x.shape
    Cr = 64  # hardcoded per spec (C=128, reduction=2)
    xt = x.transpose(0, 2, 3, 1).reshape(n, H * W, C)
    # one fused projection matmul: wcat = [w1; w2; wa] -> [2*Cr+C, C]
    qkv = xt @ wcat.T                                   # [n, HW, 2*Cr+C]
    q = _l2norm(qkv[..., :Cr] + b1) * K_AMP             # [n, HW, Cr]
    k = _l2norm(qkv[..., Cr:2 * Cr] + b2) * K_AMP
    v = qkv[..., 2 * Cr:] + ba                          # [n, HW, C]
    d = q.shape[-1]
    dn = d ** -0.25
    ratio = proj.shape[0] ** -0.5
    qd = jnp.einsum("nid,md->nim", q * dn, proj)        # [n, HW, M]
    kd = jnp.einsum("nid,md->nim", k * dn, proj)
    q_diag = jnp.sum(q * q, axis=-1, keepdims=True) * 0.5 * dn * dn
    k_diag = jnp.sum(k * k, axis=-1, keepdims=True) * 0.5 * dn * dn
    # reference takes max over the WHOLE batch of kd -> all-reduce max
    kd_max = jax.lax.pmax(jnp.max(kd), "dp")
    qp = ratio * (
        jnp.exp(qd - q_diag - jnp.max(qd, axis=-1, keepdims=True)) + EPS_KERN
    )
    kp = ratio * (jnp.exp(kd - k_diag - kd_max) + EPS_KERN)
    ksum = jnp.sum(kp, axis=1)                          # [n, M]
    ctx = jnp.einsum("nim,nie->nme", kp, v)             # [n, M, C]
    # fuse numerator (qp @ ctx) and denominator (qp @ ksum) into one matmul
    ctx_aug = jnp.concatenate([ctx, ksum[:, :, None]], axis=-1)  # [n, M, C+1]
    out_aug = jnp.einsum("nim,nme->nie", qp, ctx_aug)   # [n, HW, C+1]
    out = out_aug[..., :C] / out_aug[..., C:]
    return out.transpose(0, 2, 1).reshape(n, C, H, W) * RES_SCALE


def kernel(**inputs) -> np.ndarray:
    x = np.asarray(inputs["x"], dtype=np.float32)
    N = x.shape[0]
    per = N // N_DEV
    xs = x.reshape(N_DEV, per, *x.shape[1:])
    wcat = np.concatenate(
        [
            np.asarray(inputs["w1"], np.float32),
            np.asarray(inputs["w2"], np.float32),
            np.asarray(inputs["wa"], np.float32),
        ],
        axis=0,
    )
    out = _shard_fn(
        xs,
        jnp.asarray(wcat),
        jnp.asarray(inputs["b1"], jnp.float32),
        jnp.asarray(inputs["b2"], jnp.float32),
        jnp.asarray(inputs["ba"], jnp.float32),
        jnp.asarray(inputs["proj"], jnp.float32),
    )
    out = np.asarray(out)
    return out.reshape(N, *out.shape[2:]).astype(np.float32)
